# revision 1
# baseline (speedup 1.0000x reference)
"""DeepPot embedding kernel for Trainium2 (8 NeuronCores, SPMD).

Strategy (node-block sharding, no collectives):
  - Nodes are grouped into blocks of 128. Each of the 8 cores owns 49 blocks
    (392 blocks total incl. one dummy block past node 50000).
  - Host sorts edges by src node, deals node-blocks to cores sorted by edge
    count (slot-aligned) so every core's slot k has the same padded chunk
    count -> a single SPMD program works for all cores.
  - Per 512-edge superchunk on device:
      x^T (18,512) tiles DMA'd from host (one-hot(species[dst]) rows 0-15,
      sij row 16, ones row 17 for the bias trick),
      h = silu(W.T @ x) chain on TensorE+ScalarE (features on partitions),
      G produced edges-on-partition via flipped 4th matmul,
      M[e,(a,d)] = R[e,a]*G[e,d] on VectorE (broadcast APs),
      scatter-add GRi[n,(a,d)] += onehot(src)^T @ M on TensorE into PSUM.
  - Per block: einsum('nad,nas->nds') on VectorE, DMA out.
"""

import math
import os
import time

import numpy as np

NNODE = 50000
NEDGE = 1600000
ZMAX = 16
DIM = 64
SUBDIM = 8
HIDDEN = 64
NCORES = 8
BLK = 128          # nodes per block
GX = 8             # superchunks per x DMA group

LAST = {}          # exec metadata for test harness


# --------------------------------------------------------------------------
# Host-side preparation
# --------------------------------------------------------------------------

def _prepare(species, edge_src, edge_dst, distances, switch, vec,
             W1, b1, W2, b2, W3, b3, W4, b4, nnode, ncores):
    f32 = np.float32
    species = np.asarray(species).astype(np.int64)
    edge_src = np.asarray(edge_src).astype(np.int64)
    edge_dst = np.asarray(edge_dst).astype(np.int64)
    distances = np.asarray(distances, dtype=f32)
    switch = np.asarray(switch, dtype=f32)
    vec = np.asarray(vec, dtype=f32)
    W1 = np.asarray(W1, dtype=f32); b1 = np.asarray(b1, dtype=f32)
    W2 = np.asarray(W2, dtype=f32); b2 = np.asarray(b2, dtype=f32)
    W3 = np.asarray(W3, dtype=f32); b3 = np.asarray(b3, dtype=f32)
    W4 = np.asarray(W4, dtype=f32); b4 = np.asarray(b4, dtype=f32)

    nedge = edge_src.shape[0]
    sij = switch / distances                          # (E,)
    rinv = (1.0 / distances).astype(f32)
    r_abc = sij[:, None] * (vec * rinv[:, None])      # (E,3) = sij*vhat
    spec_d = species[edge_dst]                        # (E,)

    nblocks_real = (nnode + BLK - 1) // BLK
    nblocks = ((nblocks_real + ncores - 1) // ncores) * ncores
    nslot = nblocks // ncores

    blk_of_edge = edge_src // BLK
    counts = np.bincount(blk_of_edge, minlength=nblocks).astype(np.int64)
    eorder = np.argsort(blk_of_edge, kind="stable")
    starts = np.zeros(nblocks + 1, dtype=np.int64)
    starts[1:] = np.cumsum(counts)

    order_blocks = np.argsort(-counts, kind="stable")   # big blocks first

    # per-slot chunk count K_k (identical across cores)
    K_slots = []
    groups = []
    for k in range(nslot):
        grp = order_blocks[k * ncores:(k + 1) * ncores]
        groups.append(grp)
        K = max(1, int(math.ceil(counts[grp].max() / 128.0))) if len(grp) else 1
        K_slots.append(K)
    S_slots = [(K + 3) // 4 for K in K_slots]           # superchunks per slot
    last_subs = [K - 4 * (S - 1) for K, S in zip(K_slots, S_slots)]
    NSC = sum(S_slots)
    NSC_pad = ((NSC + GX - 1) // GX) * GX
    NCH_pad = 4 * NSC_pad

    sc_base = np.zeros(nslot + 1, dtype=np.int64)
    sc_base[1:] = np.cumsum(S_slots)

    # weights (augmented with bias rows / sij row)
    W1x = np.ascontiguousarray(
        np.concatenate([W1[1:1 + ZMAX, :], W1[0:1, :], b1[None, :]], axis=0))
    W2a = np.ascontiguousarray(W2)
    W3a = np.ascontiguousarray(W3)
    W4a = np.ascontiguousarray(np.concatenate([W4, b4[None, :]], axis=0))
    B2 = np.ascontiguousarray(b2[:, None])
    B3 = np.ascontiguousarray(b3[:, None])
    ONES = np.ones((1, 512), dtype=f32)
    IOTA = np.ascontiguousarray(
        np.broadcast_to(np.arange(128, dtype=f32)[None, :], (128, 128)))

    in_maps = []
    block_of = np.zeros((ncores, nslot), dtype=np.int64)
    for c in range(ncores):
        nslots_total = NSC * 512
        sij_s = np.zeros(nslots_total, dtype=f32)
        r1_s = np.zeros(nslots_total, dtype=f32)
        r2_s = np.zeros(nslots_total, dtype=f32)
        r3_s = np.zeros(nslots_total, dtype=f32)
        spec_s = np.full(nslots_total, -1, dtype=np.int64)
        lsrc_s = np.zeros(nslots_total, dtype=f32)

        for k in range(nslot):
            b = groups[k][c] if c < len(groups[k]) else nblocks - 1
            block_of[c, k] = b
            n = int(counts[b])
            e = eorder[starts[b]:starts[b] + n]
            base = int(sc_base[k]) * 512
            sij_s[base:base + n] = sij[e]
            r1_s[base:base + n] = r_abc[e, 0]
            r2_s[base:base + n] = r_abc[e, 1]
            r3_s[base:base + n] = r_abc[e, 2]
            spec_s[base:base + n] = spec_d[e]
            lsrc_s[base:base + n] = (edge_src[e] - b * BLK).astype(f32)

        # x tiles: (18, NSC_pad*512) feature-major
        X = np.zeros((NSC_pad, 18, 512), dtype=f32)
        X[:NSC, 16, :] = sij_s.reshape(NSC, 512)
        X[:NSC, 17, :] = 1.0
        vmask = spec_s >= 0
        vidx = np.nonzero(vmask)[0]
        X[vidx // 512, spec_s[vidx], vidx % 512] = 1.0
        Xt = np.ascontiguousarray(X.transpose(1, 0, 2).reshape(18, NSC_pad * 512))

        # r4: (128, 4*NCH_pad) lane-major, 4 values per chunk
        R4 = np.zeros((NCH_pad, 4, 128), dtype=f32)
        sl = np.arange(NSC * 512)
        ch = sl // 128
        ln = sl % 128
        R4[ch, 0, ln] = sij_s
        R4[ch, 1, ln] = r1_s
        R4[ch, 2, ln] = r2_s
        R4[ch, 3, ln] = r3_s
        R4t = np.ascontiguousarray(R4.transpose(2, 0, 1).reshape(128, NCH_pad * 4))

        L = np.zeros((NCH_pad, 128), dtype=f32)
        L[ch, ln] = lsrc_s
        Lt = np.ascontiguousarray(L.transpose(1, 0))

        in_maps.append({
            "x": Xt, "r4": R4t, "lsrc": Lt,
            "w1": W1x, "w2": W2a, "w3": W3a, "w4": W4a, "iota": IOTA,
            "b2": B2, "b3": B3, "ones": ONES,
        })

    plan = {
        "nslot": nslot, "S_slots": S_slots, "last_subs": last_subs,
        "NSC": NSC, "NSC_pad": NSC_pad, "NCH_pad": NCH_pad,
        "block_of": block_of, "nblocks_real": nblocks_real,
    }
    return in_maps, plan


# --------------------------------------------------------------------------
# Device program
# --------------------------------------------------------------------------

def _build(plan, use_f32r=True, sim_safe=False, variant='full', reps=1, pipe=False, bf16sc=False):
    import concourse.bass as bass
    import concourse.tile as tile
    from concourse import bacc, mybir

    F32 = mybir.dt.float32
    F32R = mybir.dt.float32r
    AF = mybir.ActivationFunctionType
    OP = mybir.AluOpType

    DT = F32R if use_f32r else F32
    BF16 = mybir.dt.bfloat16
    DTS = BF16 if bf16sc else DT      # scatter-path dtype (h3s/w4/oh/mt)

    nslot = plan["nslot"]
    S_slots = plan["S_slots"]
    last_subs = plan["last_subs"]
    NSC_pad = plan["NSC_pad"]
    NCH_pad = plan["NCH_pad"]

    nc = bacc.Bacc("TRN2", target_bir_lowering=False, debug=False)

    xd = nc.dram_tensor("x", [18, NSC_pad * 512], DT, kind="ExternalInput")
    r4d = nc.dram_tensor("r4", [128, NCH_pad * 4], F32, kind="ExternalInput")
    lsd = nc.dram_tensor("lsrc", [128, NCH_pad], F32, kind="ExternalInput")
    w1d = nc.dram_tensor("w1", [18, 64], DT, kind="ExternalInput")
    w2d = nc.dram_tensor("w2", [64, 64], DT, kind="ExternalInput")
    w3d = nc.dram_tensor("w3", [64, 64], DT, kind="ExternalInput")
    b2d = nc.dram_tensor("b2", [64, 1], F32, kind="ExternalInput")
    b3d = nc.dram_tensor("b3", [64, 1], F32, kind="ExternalInput")
    onesd = nc.dram_tensor("ones", [1, 512], DTS, kind="ExternalInput")
    w4d = nc.dram_tensor("w4", [65, 64], DTS, kind="ExternalInput")
    iod = nc.dram_tensor("iota", [128, 128], F32, kind="ExternalInput")
    outd = nc.dram_tensor("out", [nslot * 128, 512], F32, kind="ExternalOutput")

    with tile.TileContext(nc) as tc:
        from contextlib import ExitStack
        with ExitStack() as ctx:
            const = ctx.enter_context(tc.tile_pool(name="const", bufs=1))
            hstat = ctx.enter_context(tc.tile_pool(name="hstat", bufs=1))
            _b = (lambda n: n + 2) if pipe else (lambda n: n)
            xpool = ctx.enter_context(tc.tile_pool(name="xpool", bufs=_b(4)))
            r4pool = ctx.enter_context(tc.tile_pool(name="r4pool", bufs=_b(3)))
            lspool = ctx.enter_context(tc.tile_pool(name="lspool", bufs=_b(3)))
            ohpool = ctx.enter_context(tc.tile_pool(name="ohpool", bufs=_b(3)))
            mpool = ctx.enter_context(tc.tile_pool(name="mpool", bufs=_b(3)))
            gripool = ctx.enter_context(
                tc.tile_pool(name="gripool", bufs=2, space=bass.MemorySpace.PSUM))
            hppool = ctx.enter_context(
                tc.tile_pool(name="hppool", bufs=(4 if pipe else 1),
                             space=bass.MemorySpace.PSUM))
            g4pool = ctx.enter_context(
                tc.tile_pool(name="g4pool", bufs=2, space=bass.MemorySpace.PSUM))
            grispool = ctx.enter_context(tc.tile_pool(name="grispool", bufs=2))
            embpool = ctx.enter_context(tc.tile_pool(name="embpool", bufs=2))
            einpool = ctx.enter_context(tc.tile_pool(name="einpool", bufs=2))
            sigpool = (ctx.enter_context(tc.tile_pool(name="sigpool", bufs=2))
                       if sim_safe else None)

            def silu_act(dst, src, bias=0.0):
                # dst: SBUF (64,512) slice; src: PSUM (64,512)
                if not sim_safe:
                    nc.scalar.activation(dst, src, AF.Silu, bias=bias)
                else:
                    sg = sigpool.tile([64, 512], F32, tag="sig")
                    nc.scalar.activation(sg[:, :], src, AF.Sigmoid, bias=bias)
                    nc.vector.scalar_tensor_tensor(
                        dst, src, bias, sg[:, :], OP.add, OP.mult)

            # constants
            w1t = const.tile([18, 64], DT, tag="w1t")
            nc.sync.dma_start(out=w1t[:, :], in_=w1d[:, :])
            w2t = const.tile([64, 64], DT, tag="w2t")
            nc.sync.dma_start(out=w2t[:, :], in_=w2d[:, :])
            w3t = const.tile([64, 64], DT, tag="w3t")
            nc.sync.dma_start(out=w3t[:, :], in_=w3d[:, :])
            b2t = const.tile([64, 1], F32, tag="b2t")
            nc.sync.dma_start(out=b2t[:, :], in_=b2d[:, :])
            b3t = const.tile([64, 1], F32, tag="b3t")
            nc.sync.dma_start(out=b3t[:, :], in_=b3d[:, :])
            w4t = const.tile([65, 64], DTS, tag="w4t")
            nc.sync.dma_start(out=w4t[:, :], in_=w4d[:, :])
            iot = const.tile([128, 128], F32, tag="iot")
            nc.sync.dma_start(out=iot[:, :], in_=iod[:, :])

            # static double-buffered h tiles; h3s keeps a persistent ones
            # row (row 64, DMA'd once) for the W4 bias trick
            hset = []
            for p in range(3 if pipe else 2):
                t1 = hstat.tile([64, 512], DT, tag=f"h0s{p}")
                t2 = hstat.tile([64, 512], DT, tag=f"h1s{p}")
                t3 = hstat.tile([65, 512], DTS, tag=f"h2s{p}")
                nc.sync.dma_start(out=t3[64:65, :], in_=onesd[:, :])
                hset.append([t1, t2, t3])

            from contextlib import nullcontext
            loop_cm = tc.For_i(0, reps, 1) if reps > 1 else nullcontext()
            with loop_cm:
                sc_flat = 0
                xg = r4g = lsg = None
                for k in range(nslot):
                    S = S_slots[k]
                    gri = gripool.tile([128, 256], F32, tag="gri")
                    first_mm = True
                    for j in range(S):
                        subs = 4 if j < S - 1 else last_subs[k]
                        if sc_flat % GX == 0:
                            gbase = sc_flat
                            xg = xpool.tile([18, GX * 512], DT, tag="xg")
                            nc.sync.dma_start(
                                out=xg[:, :],
                                in_=xd[:, gbase * 512:(gbase + GX) * 512])
                            r4g = r4pool.tile([128, GX * 16], F32, tag="r4g")
                            nc.sync.dma_start(
                                out=r4g[:, :],
                                in_=r4d[:, gbase * 16:(gbase + GX) * 16])
                            lsg = lspool.tile([128, GX * 4], F32, tag="lsg")
                            nc.sync.dma_start(
                                out=lsg[:, :],
                                in_=lsd[:, gbase * 4:(gbase + GX) * 4])
                        o = sc_flat - gbase
                        xsc = xg[:, o * 512:(o + 1) * 512]
                        h1s, h2s, h3s = hset[sc_flat % len(hset)]

                        do_mlp = variant in ('full', 'mlponly', 'noein')
                        do_scatter = variant in ('full', 'scatteronly', 'noein')
                        do_ein = variant in ('full', 'mlponly', 'scatteronly')
                        h1p = hppool.tile([64, 512], F32, tag="hp" if pipe else "h1p")
                        if do_mlp:
                            nc.tensor.matmul(h1p[:, :], w1t[:, :], xsc,
                                             start=True, stop=True)
                            silu_act(h1s[0:64, :], h1p[:, :])

                            h2p = hppool.tile([64, 512], F32, tag="hp" if pipe else "h2p")
                            nc.tensor.matmul(h2p[:, :], w2t[:, :], h1s[0:64, :],
                                             start=True, stop=True)
                            silu_act(h2s[0:64, :], h2p[:, :], bias=b2t[:, 0:1])

                            h3p = hppool.tile([64, 512], F32, tag="hp" if pipe else "h3p")
                            nc.tensor.matmul(h3p[:, :], w3t[:, :], h2s[0:64, :],
                                             start=True, stop=True)
                            silu_act(h3s[0:64, :], h3p[:, :], bias=b3t[:, 0:1])

                        g4 = g4pool.tile([128, 256], F32, tag="g4")
                        if do_scatter:
                            for cs in range(subs):
                                nc.tensor.matmul(
                                    g4[:, cs * 64:(cs + 1) * 64],
                                    h3s[:, cs * 128:(cs + 1) * 128],
                                    w4t[:, :],
                                    start=True, stop=True)

                        for cs in range(subs if do_scatter else 0):
                            chn = o * 4 + cs           # chunk index within group
                            oh = ohpool.tile([128, 128], DTS, tag="oh")
                            nc.gpsimd.tensor_scalar(
                                oh[:, :], iot[:, :],
                                lsg[:, chn:chn + 1], None, OP.is_equal)
                            mt = mpool.tile([128, 256], DTS, tag="mt")
                            g4s = g4[:, cs * 64:(cs + 1) * 64]
                            in0 = g4s.unsqueeze(1).broadcast_to((128, 4, 64))
                            r4c = r4g[:, chn * 4:chn * 4 + 4]
                            in1 = r4c.unsqueeze(2).broadcast_to((128, 4, 64))
                            mt3 = mt[:, :].rearrange("p (a d) -> p a d", a=4)
                            nc.vector.tensor_tensor(mt3, in0, in1, OP.mult)
                            is_last = (j == S - 1) and (cs == subs - 1)
                            nc.tensor.matmul(gri[:, :], oh[:, :], mt[:, :],
                                             start=first_mm, stop=is_last)
                            first_mm = False
                        sc_flat += 1

                    # ---- block epilogue: einsum + store ----
                    gris = grispool.tile([128, 256], F32, tag="gris")
                    nc.vector.tensor_copy(gris[:, :], gri[:, :])
                    emb = embpool.tile([128, 512], F32, tag="emb")
                    emb3 = emb[:, :].rearrange("p (d s) -> p d s", s=8)
                    for s in range(8 if do_ein else 0):
                        t1 = einpool.tile([128, 64], F32, tag="ein1")
                        nc.vector.tensor_scalar(
                            t1[:, :], gris[:, 0:64],
                            gris[:, s:s + 1], None, OP.mult)
                        t2 = einpool.tile([128, 64], F32, tag="ein2")
                        nc.vector.scalar_tensor_tensor(
                            t2[:, :], gris[:, 64:128], gris[:, 64 + s:65 + s],
                            t1[:, :], OP.mult, OP.add)
                        t3 = einpool.tile([128, 64], F32, tag="ein3")
                        nc.vector.scalar_tensor_tensor(
                            t3[:, :], gris[:, 128:192], gris[:, 128 + s:129 + s],
                            t2[:, :], OP.mult, OP.add)
                        outs = emb3[:, :, s:s + 1].squeeze(2)
                        nc.vector.scalar_tensor_tensor(
                            outs, gris[:, 192:256], gris[:, 192 + s:193 + s],
                            t3[:, :], OP.mult, OP.add)
                    nc.sync.dma_start(out=outd[k * 128:(k + 1) * 128, :],
                                      in_=emb[:, :])

    nc.compile()
    return nc


# --------------------------------------------------------------------------
# Entry point
# --------------------------------------------------------------------------

def _measure(plan, in_maps, build_kw, ncal=6, r2=9):
    """HW timing via reps-loop differencing: build the same kernel with a
    hardware For_i repeat of 1 and r2, difference median call times."""
    import statistics

    import jax

    fns = {}
    for r in (1, r2):
        nc = _build(plan, reps=r, **build_kw)
        fns[r] = _build_fn(nc, in_maps)
        jax.block_until_ready(fns[r][0](*fns[r][1]))
    ts = {1: [], r2: []}
    for _ in range(ncal):
        for r in (1, r2):
            fn, bufs = fns[r]
            t0 = time.time()
            jax.block_until_ready(fn(*bufs))
            ts[r].append(time.time() - t0)
    m1 = statistics.median(ts[1])
    m2 = statistics.median(ts[r2])
    LAST["measure_times"] = {1: ts[1], r2: ts[r2]}
    return (m2 - m1) / (r2 - 1) * 1e9


def _build_fn(nc, in_maps):
    import jax
    from jax.experimental.shard_map import shard_map
    from jax.sharding import Mesh, PartitionSpec

    from concourse import mybir
    from concourse.bass2jax import (_bass_exec_p, install_neuronx_cc_hook,
                                    partition_id_tensor)

    install_neuronx_cc_hook()
    partition_name = (nc.partition_id_tensor.name
                      if nc.partition_id_tensor else None)
    in_names, out_names, out_avals = [], [], []
    for alloc in nc.m.functions[0].allocations:
        if not isinstance(alloc, mybir.MemoryLocationSet):
            continue
        name = alloc.memorylocations[0].name
        if alloc.kind == "ExternalInput":
            if name != partition_name:
                in_names.append(name)
        elif alloc.kind == "ExternalOutput":
            out_names.append(name)
            out_avals.append(jax.core.ShapedArray(
                tuple(alloc.tensor_shape), mybir.dt.np(alloc.dtype)))
    n_params = len(in_names)
    all_in_names = in_names + out_names
    if partition_name is not None:
        all_in_names.append(partition_name)

    def _body(*args):
        extra = ([partition_id_tensor()] if partition_name is not None else [])
        outs = _bass_exec_p.bind(
            *args, *extra,
            out_avals=tuple(out_avals), in_names=tuple(all_in_names),
            out_names=tuple(out_names), lowering_input_output_aliases=(),
            sim_require_finite=True, sim_require_nnan=True, nc=nc)
        return tuple(outs)

    devices = jax.devices()[:NCORES]
    mesh = Mesh(np.asarray(devices), ("core",))
    nin = n_params + len(out_names)
    concat_in = [np.concatenate([np.asarray(m[n]) for m in in_maps], axis=0)
                 for n in in_names]
    concat_zeros = [np.zeros((NCORES * a.shape[0], *a.shape[1:]), a.dtype)
                    for a in out_avals]
    sharding = jax.sharding.NamedSharding(mesh, PartitionSpec("core"))
    bufs = [jax.device_put(a, sharding) for a in concat_in + concat_zeros]
    fn = jax.jit(shard_map(
        _body, mesh=mesh, in_specs=(PartitionSpec("core"),) * nin,
        out_specs=(PartitionSpec("core"),) * len(out_names), check_rep=False))
    return fn, bufs


BUILD_KW = {"pipe": False, "bf16sc": False}


def _adapt_maps(in_maps, build_kw):
    if build_kw.get("bf16sc"):
        import ml_dtypes
        bf = ml_dtypes.bfloat16
        in_maps = [dict(m, w4=m["w4"].astype(bf), ones=m["ones"].astype(bf))
                   for m in in_maps]
    return in_maps


def kernel(**inputs):
    from concourse.bass_utils import run_bass_kernel_spmd

    in_maps, plan = _prepare(nnode=NNODE, ncores=NCORES, **inputs)
    in_maps = _adapt_maps(in_maps, BUILD_KW)
    t0 = time.time()
    nc = _build(plan, **BUILD_KW)
    t1 = time.time()
    res = run_bass_kernel_spmd(nc, in_maps, list(range(NCORES)), trace=False)
    t2 = time.time()
    LAST["build_s"] = t1 - t0
    LAST["run_s"] = t2 - t1
    LAST["exec_time_ns"] = res.exec_time_ns
    if os.environ.get("KMEASURE", "") == "1":
        try:
            LAST["exec_time_ns"] = _measure(plan, in_maps, BUILD_KW)
        except Exception as e:  # measurement is best-effort
            LAST["measure_error"] = repr(e)

    nslot = plan["nslot"]
    block_of = plan["block_of"]
    nb_real = plan["nblocks_real"]
    nblocks = ((nb_real + NCORES - 1) // NCORES) * NCORES
    emb_full = np.zeros((nblocks * BLK, 512), dtype=np.float32)
    for c in range(NCORES):
        oc = res.results[c]["out"]
        for k in range(nslot):
            b = int(block_of[c, k])
            emb_full[b * BLK:(b + 1) * BLK, :] = oc[k * 128:(k + 1) * 128, :]
    return emb_full[:NNODE, :]


# --------------------------------------------------------------------------
# Small-scale numpy reference + CoreSim self-test (dev only)
# --------------------------------------------------------------------------

def _np_reference(species, edge_src, edge_dst, distances, switch, vec,
                  W1, b1, W2, b2, W3, b3, W4, b4, nnode):
    f32 = np.float32
    def silu(x):
        return x / (1.0 + np.exp(-x))
    onehot = np.eye(ZMAX, dtype=f32)[np.asarray(species, np.int64)]
    d = np.asarray(distances, f32)[:, None]
    sw = np.asarray(switch, f32)[:, None]
    vhat = np.asarray(vec, f32) / d
    sij = sw / d
    Rij = np.concatenate((sij, sij * vhat), axis=-1)
    x = np.concatenate((sij, onehot[np.asarray(edge_dst, np.int64)]), axis=-1)
    h = silu(x @ W1 + b1)
    h = silu(h @ W2 + b2)
    h = silu(h @ W3 + b3)
    Gij = h @ W4 + b4
    GRi = np.zeros((nnode, 4, Gij.shape[1]), f32)
    np.add.at(GRi, np.asarray(edge_src, np.int64),
              Gij[:, None, :] * Rij[:, :, None])
    GRisub = GRi[:, :, :SUBDIM]
    return np.einsum('nad,nas->nds', GRi, GRisub).reshape(nnode, -1)


def _selftest(nnode=1024, nedge=16000, ncores=2, seed=0):
    from concourse.bass_interp import CoreSim
    rng = np.random.default_rng(seed)
    f32 = np.float32
    ins = dict(
        species=rng.integers(0, ZMAX, nnode),
        edge_src=rng.integers(0, nnode, nedge),
        edge_dst=rng.integers(0, nnode, nedge),
        distances=(rng.random(nedge, dtype=f32) * 4.5 + 0.5),
        switch=rng.random(nedge, dtype=f32),
        vec=rng.standard_normal((nedge, 3), dtype=f32),
        W1=rng.standard_normal((1 + ZMAX, HIDDEN), dtype=f32) / 4,
        b1=np.zeros(HIDDEN, f32),
        W2=rng.standard_normal((HIDDEN, HIDDEN), dtype=f32) / 8,
        b2=np.zeros(HIDDEN, f32),
        W3=rng.standard_normal((HIDDEN, HIDDEN), dtype=f32) / 8,
        b3=np.zeros(HIDDEN, f32),
        W4=rng.standard_normal((HIDDEN, DIM), dtype=f32) / 8,
        b4=np.zeros(DIM, f32),
    )
    expected = _np_reference(nnode=nnode, **ins)
    in_maps, plan = _prepare(nnode=nnode, ncores=ncores, **ins)
    print("plan: NSC", plan["NSC"], "S_slots", plan["S_slots"][:6], "...")
    nc = _build(plan, use_f32r=True, sim_safe=True)
    nslot = plan["nslot"]
    emb_full = np.zeros((plan["block_of"].max() * BLK + BLK, 512), np.float32)
    for c in range(ncores):
        sim = CoreSim(nc, trace=False)
        for name, arr in in_maps[c].items():
            sim.tensor(name)[:] = arr
        sim.simulate()
        oc = np.array(sim.tensor("out"))
        for k in range(nslot):
            b = int(plan["block_of"][c, k])
            emb_full[b * BLK:(b + 1) * BLK, :] = oc[k * 128:(k + 1) * 128, :]
    actual = emb_full[:nnode, :]
    err = np.linalg.norm(actual - expected) / max(np.linalg.norm(expected), 1e-30)
    print("selftest rel fro err:", err)
    amax = np.max(np.abs(actual - expected))
    print("selftest max abs err:", amax, "scale", np.max(np.abs(expected)))
    return err


if __name__ == "__main__":
    _selftest()

